# revision 1
# baseline (speedup 1.0000x reference)
"""MixedScoreMultiHeadAttention Trainium2 kernel (PE-centric pipeline).

Data-parallel over batch: 32 batches -> 8 cores x 4 batches.

Per (b):
  dot_h = q_h k_h^T  (per head, PE)  -> flattened r-major into rhs rows
  layer1: T[(h,m), pts] = a[h,m]*dot_h[pts] + c[h,m]*Y[pts]   (PE matmul,
          constant stationary [17,128]; bias b[h,m] folded into relu evac)
  R = relu(T + b)                     (ACT/DVE evacuation from PSUM, fp16)
  layer2: mixed^T[pts, h] via stationary-swapped matmul (lhsT = R data,
          rhs = block-diag w2 [128,8]) -> PSUM [c, (r-grp, h)] full-partition
  exp-evac (ACT Exp) -> w_sb [c, (r,h)] fp32
  AV: out[r, 17] = w^T-slice.T @ [v|1]  (ones col gives softmax denominator)
  normalize by reciprocal of col 16 -> out rows

mix2 bias b2 is dropped (constant shift is softmax-invariant); 1/sqrt(D) is
folded into Wq host-side.
"""
import sys

sys.path.insert(0, "/opt/trn_rl_repo")

import numpy as np
from contextlib import ExitStack

import concourse.bass as bass
import concourse.mybir as mybir
import concourse.tile as tile
from concourse import bacc
from concourse.bass_utils import run_bass_kernel_spmd
from concourse.masks import make_identity

B, R, C, E, H, D, MS = 32, 128, 128, 256, 16, 16, 16
NCORES = 8
BL = B // NCORES  # batches per core: 4
TOK = BL * R      # 512 tokens per core per side
PTS = R * C       # 16384 score points per (b)

FP32 = mybir.dt.float32
FP16 = mybir.dt.float16
AF = mybir.ActivationFunctionType
ALU = mybir.AluOpType



def build_kernel():
    nc = bacc.Bacc("TRN2", target_bir_lowering=False, debug=False,
                   num_devices=NCORES)

    x_r = nc.dram_tensor("x_r", [TOK, E], FP32, kind="ExternalInput").ap()
    x_c = nc.dram_tensor("x_c", [TOK, E], FP32, kind="ExternalInput").ap()
    cost = nc.dram_tensor("cost", [BL, R, C], FP32, kind="ExternalInput").ap()
    # Wq pre-scaled by 1/sqrt(D) host-side; head-padding to 32-col slots
    # (for 32-aligned projection PSUM rows) happens on-chip.
    wq_d = nc.dram_tensor("Wq", [E, E], FP32, kind="ExternalInput").ap()
    wk_d = nc.dram_tensor("Wk", [E, E], FP32, kind="ExternalInput").ap()
    wv_d = nc.dram_tensor("Wv", [E, E], FP32, kind="ExternalInput").ap()
    # layer1 stationary [17, 256]: col (half*128 + (h%8)*16 + m):
    #   row h' = a[h,m] iff h'==h; row 16 = c[h,m]
    w1_d = nc.dram_tensor("W1L", [17, 2 * 128], FP32,
                          kind="ExternalInput").ap()
    # layer2 moving [128, 16]: col (half*8 + j): row hm = w2[half*8+j, m]
    # iff hm == ((j)*16+m) else 0
    w2_d = nc.dram_tensor("W2L", [128, 16], FP32, kind="ExternalInput").ap()
    # relu bias per (h,m) row: bcol2[hm, half] = b1[half*8 + hm//16, hm%16]
    bc_d = nc.dram_tensor("bcol2", [128, 2], FP32, kind="ExternalInput").ap()
    out_d = nc.dram_tensor("out", [BL, R, H * D], FP32,
                           kind="ExternalOutput").ap()

    with tile.TileContext(nc) as tc, ExitStack() as ctx:
        const_p = ctx.enter_context(tc.tile_pool(name="const", bufs=1))
        inx_p = ctx.enter_context(tc.tile_pool(name="inx", bufs=2))
        w_p = ctx.enter_context(tc.tile_pool(name="wts", bufs=1))
        xt_p = ctx.enter_context(tc.tile_pool(name="xt", bufs=1))
        qkv_p = ctx.enter_context(tc.tile_pool(name="qkv", bufs=1))
        x4_p = ctx.enter_context(tc.tile_pool(name="x4", bufs=1))
        rhs_p = ctx.enter_context(tc.tile_pool(name="rhs", bufs=2))
        rr_p = ctx.enter_context(tc.tile_pool(name="rr", bufs=6))
        wsb_p = ctx.enter_context(tc.tile_pool(name="wsb", bufs=2))
        fout_p = ctx.enter_context(tc.tile_pool(name="fout", bufs=1))
        small_p = ctx.enter_context(tc.tile_pool(name="small", bufs=4))
        ps_tr = ctx.enter_context(
            tc.tile_pool(name="pstr", bufs=1, space="PSUM"))
        ps_big = ctx.enter_context(
            tc.tile_pool(name="psb", bufs=4, space="PSUM"))
        ps_l2 = ctx.enter_context(
            tc.tile_pool(name="psl2", bufs=2, space="PSUM"))
        ps_av = ctx.enter_context(
            tc.tile_pool(name="psa", bufs=1, space="PSUM"))

        ident = const_p.tile([128, 128], FP32)
        make_identity(nc, ident[:])

        # ---- small weight/const loads
        w1f = inx_p.tile([17, 2 * 128], FP32, tag="w1f")
        nc.sync.dma_start(w1f[:], w1_d[:])
        w1l = const_p.tile([17, 2 * 128], FP16)
        nc.vector.tensor_copy(w1l[:], w1f[:])

        w2f = inx_p.tile([128, 16], FP32, tag="w2f")
        nc.sync.dma_start(w2f[:], w2_d[:])
        w2l = const_p.tile([128, 16], FP16)
        nc.vector.tensor_copy(w2l[:], w2f[:])

        bcol2 = const_p.tile([128, 2], FP32)
        nc.sync.dma_start(bcol2[:], bc_d[:])

        # ---- QKV weights fp16 (q/k padded on-chip: head h -> 32-col slot)
        wt16 = {}
        for name, dram in (("q", wq_d), ("k", wk_d), ("v", wv_d)):
            halves = []
            for eh in range(2):
                w32 = inx_p.tile([128, E], FP32, tag="wload")
                nc.sync.dma_start(w32[:], dram[eh * 128:(eh + 1) * 128, :])
                ncols = E if name == "v" else 2 * E
                w16 = w_p.tile([128, ncols], FP16, tag=f"w16{name}{eh}",
                               name=f"w16{name}{eh}")
                if name == "v":
                    nc.vector.tensor_copy(w16[:], w32[:])
                else:
                    nc.gpsimd.memset(w16[:], 0.0)
                    w16v = w16[:].rearrange("p (h x) -> p h x", h=H)
                    w32v = w32[:].rearrange("p (h x) -> p h x", h=H)
                    nc.vector.tensor_copy(w16v[:, :, 0:D], w32v[:])
                halves.append(w16)
            wt16[name] = halves

        # ---- x load + PE transpose -> xT fp16 [2 e-halves][128, TOK]
        xT = {}
        for name, dram in (("r", x_r), ("c", x_c)):
            xt0 = xt_p.tile([128, TOK], FP16, tag=f"xT{name}0")
            xt1 = xt_p.tile([128, TOK], FP16, tag=f"xT{name}1")
            xT[name] = [xt0, xt1]
            for t in range(BL):
                x32 = inx_p.tile([128, E], FP32, tag="xload")
                nc.sync.dma_start(x32[:], dram[t * 128:(t + 1) * 128, :])
                for eh in range(2):
                    pst = ps_tr.tile([128, 128], FP32, tag="pstr")
                    nc.tensor.transpose(
                        pst[:], x32[:, eh * 128:(eh + 1) * 128], ident[:])
                    nc.vector.tensor_copy(
                        xT[name][eh][:, t * 128:(t + 1) * 128], pst[:])

        # ---- cost -> fp16 [r, c] tiles (r-major flatten later)
        y16 = []
        for b in range(BL):
            c32 = inx_p.tile([128, C], FP32, tag="cload")
            nc.sync.dma_start(c32[:], cost[b])
            y1 = const_p.tile([128, C], FP16, name=f"y16_{b}", tag=f"y16_{b}")
            nc.vector.tensor_copy(y1[:], c32[:])
            y16.append(y1)

        # ---- projections: qT/kT per-head tiles [16, TOK] fp16
        qT, kT = [], []
        for proj, dst in (("q", qT), ("k", kT)):
            for mh in range(4):  # head-quad tiles (4 heads x 32 rows)
                ps = ps_big.tile([128, TOK], FP32, tag="psbig")
                for eh in range(2):
                    nc.tensor.matmul(
                        ps[:],
                        wt16[proj][eh][:, mh * 128:(mh + 1) * 128],
                        xT["r" if proj == "q" else "c"][eh][:],
                        start=(eh == 0), stop=(eh == 1))
                # pack 3 head evacs in one 96-row op (PE matmul operands may
                # sit at base partition 0/32/64; 96 is invalid -> separate)
                quad = qkv_p.tile([96, TOK], FP16, tag=f"{proj}Q{mh}",
                                  name=f"{proj}Q{mh}")
                last = qkv_p.tile([16, TOK], FP16, tag=f"{proj}L{mh}",
                                  name=f"{proj}L{mh}")
                if mh % 2 == 0:
                    nc.scalar.copy(quad[:], ps[0:96, :])
                    nc.vector.tensor_copy(last[:], ps[96:112, :])
                else:
                    nc.vector.tensor_copy(quad[:], ps[0:96, :])
                    nc.scalar.copy(last[:], ps[96:112, :])
                for hh in range(4):
                    dst.append(quad[hh * 32:hh * 32 + 16, :] if hh < 3
                               else last[:])

        # ---- v natural [c, hd] fp32 interleaved with ones col -> vhat
        vhat = []
        for b in range(BL):
            vh = qkv_p.tile([128, 17 * H], FP32, tag=f"vhat{b}",
                            name=f"vhat{b}")
            vh3 = vh[:].rearrange("p (h x) -> p h x", h=H)
            nc.gpsimd.memset(vh3[:, :, 16:17], 1.0)
            ps = ps_big.tile([128, E], FP32, tag="psbig")
            for eh in range(2):
                nc.tensor.matmul(
                    ps[:], xT["c"][eh][:, b * 128:(b + 1) * 128],
                    wt16["v"][eh][:], start=(eh == 0), stop=(eh == 1))
            nc.scalar.copy(
                vh3[:, :, 0:16], ps[:].rearrange("p (h x) -> p h x", h=H))
            vhat.append(vh)

        # ---- dots: X4[h] fp16 [r, (b, c)]
        x4s = []
        for h in range(H):
            x4 = x4_p.tile([128, BL * C], FP16, tag=f"x4_{h}",
                           name=f"x4_{h}")
            psd = ps_big.tile([128, BL * C], FP32, tag="psbig")
            for b in range(BL):
                nc.tensor.matmul(
                    psd[:, b * 128:(b + 1) * 128],
                    qT[h][:, b * 128:(b + 1) * 128],
                    kT[h][:, b * 128:(b + 1) * 128])
            if h % 2 == 0:
                nc.scalar.copy(x4[:], psd[:])
            else:
                nc.vector.tensor_copy(x4[:], psd[:])
            x4s.append(x4)

        # ---- per (b): layer1+relu, layer2, exp, AV
        fouts = [fout_p.tile([128, H * D], FP32, tag=f"fo{b}", name=f"fo{b}")
                 for b in range(BL)]
        for b in range(BL):
            rhs = rhs_p.tile([17, PTS], FP16, tag="rhs")
            for h in range(H):
                nc.sync.dma_start(rhs[h:h + 1, :],
                                  x4s[h][:, b * 128:(b + 1) * 128])
            nc.sync.dma_start(rhs[16:17, :], y16[b][:])

            for half in range(2):
                wsb = wsb_p.tile([128, 8 * C], FP32)
                for grp in range(2):  # 64 r's per group
                    ps2 = ps_l2.tile([128, 512], FP32)
                    for cki in range(16):  # layer1 chunks of 512 pts
                        ck = grp * 16 + cki
                        rr = rr_p.tile([128, 512], FP16, tag="rr")
                        ps1 = ps_big.tile([128, 512], FP32, tag="psbig")
                        nc.tensor.matmul(
                            ps1[:], w1l[:, half * 128:(half + 1) * 128],
                            rhs[:, ck * 512:(ck + 1) * 512])
                        if ck % 2 == 0:
                            nc.scalar.activation(
                                rr[:], ps1[:], AF.Relu,
                                bias=bcol2[:, half:half + 1])
                        else:
                            nc.vector.tensor_scalar(
                                rr[:], ps1[:], bcol2[:, half:half + 1],
                                0.0, ALU.add, ALU.max)
                        for s in range(4):  # layer2 per 128-pt subchunk
                            rloc = cki * 4 + s
                            nc.tensor.matmul(
                                ps2[:, rloc * 8:rloc * 8 + 8],
                                rr[:, s * 128:(s + 1) * 128],
                                w2l[:, half * 8:(half + 1) * 8])
                    nc.scalar.activation(
                        wsb[:, grp * 512:(grp + 1) * 512], ps2[:], AF.Exp)

                # AV + normalize for the 8 heads of this half
                psa = ps_av.tile([128, 17 * 8], FP32)
                wsb4 = wsb[:].rearrange("p (g s h) -> p g s h", g=2, s=64)
                for hl in range(8):
                    h = half * 8 + hl
                    nc.tensor.matmul(
                        psa[:, hl * 17:(hl + 1) * 17],
                        wsb4[:, :, :, hl],
                        vhat[b][:, h * 17:(h + 1) * 17])
                rec = small_p.tile([128, 8], FP32, tag="rec")
                psa3 = psa[:].rearrange("p (x y) -> p x y", x=8)
                nc.vector.reciprocal(rec[:], psa3[:, :, 16])
                for hl in range(8):
                    h = half * 8 + hl
                    nc.vector.tensor_scalar(
                        fouts[b][:, h * D:(h + 1) * D], psa3[:, hl, 0:16],
                        rec[:, hl:hl + 1], None, ALU.mult)

        for b in range(BL):
            nc.sync.dma_start(out_d[b], fouts[b][:])

    nc.compile()
    return nc


_cache = {}


def kernel(**inputs):
    row_emb = np.asarray(inputs["row_emb"], dtype=np.float32)
    col_emb = np.asarray(inputs["col_emb"], dtype=np.float32)
    cost_mat = np.asarray(inputs["cost_mat"], dtype=np.float32)
    Wq = np.asarray(inputs["Wq"], dtype=np.float32)
    Wk = np.asarray(inputs["Wk"], dtype=np.float32)
    Wv = np.asarray(inputs["Wv"], dtype=np.float32)
    m1w = np.asarray(inputs["mix1_weight"], dtype=np.float32)
    m1b = np.asarray(inputs["mix1_bias"], dtype=np.float32)
    m2w = np.asarray(inputs["mix2_weight"], dtype=np.float32)

    a1 = m1w[:, 0, :]
    c1 = m1w[:, 1, :]
    w2 = m2w[:, :, 0]

    if "nc" not in _cache:
        _cache["nc"] = build_kernel()
    nc = _cache["nc"]

    wq_s = Wq * (1.0 / np.sqrt(D))
    wk_p = Wk

    w1l = np.zeros((17, 256), dtype=np.float32)
    w2l = np.zeros((128, 16), dtype=np.float32)
    bcol2 = np.zeros((128, 2), dtype=np.float32)
    for h in range(H):
        half, hl = h // 8, h % 8
        for m in range(MS):
            col = half * 128 + hl * 16 + m
            w1l[h, col] = a1[h, m]
            w1l[16, col] = c1[h, m]
            w2l[hl * 16 + m, half * 8 + hl] = w2[h, m]
            bcol2[hl * 16 + m, half] = m1b[h, m]

    in_maps = []
    for i in range(NCORES):
        sl = slice(i * BL, (i + 1) * BL)
        in_maps.append({
            "x_r": row_emb[sl].reshape(TOK, E),
            "x_c": col_emb[sl].reshape(TOK, E),
            "cost": cost_mat[sl],
            "Wq": wq_s, "Wk": wk_p, "Wv": Wv,
            "W1L": w1l, "W2L": w2l, "bcol2": bcol2,
        })
    res = run_bass_kernel_spmd(nc, in_maps, list(range(NCORES)))
    out = np.concatenate([res.results[i]["out"] for i in range(NCORES)],
                         axis=0)
    return out.astype(np.float32)



# revision 2
# speedup vs baseline: 1.0132x; 1.0132x over previous
"""MixedScoreMultiHeadAttention Trainium2 kernel.

v3 + DMA-count reduction (the v3 profile showed 45+us of serialized HWDGE
issue overhead from 89 DMAs at ~625ns each):
  - all fp16 inputs packed host-side into one blob -> 2 input DMAs
  - dots staged through a DRAM bounce: one SBUF->DRAM dump, then ONE
    DRAM->SBUF gather per batch (h-major on both sides, so ordering works;
    a direct SBUF->SBUF multi-row gather iterates the source r-major and
    scrambles)
  - cost hi+res combined into one [BL, 2, R, C] tensor -> 1 DMA per batch
  - single fouts tile + one output DMA (DRAM side reordered r-major)
Also: projection/dot evacuations split across ACT/DVE so ACT is not idle
during warmup.
"""
import sys

sys.path.insert(0, "/opt/trn_rl_repo")

import numpy as np
import ml_dtypes
from contextlib import ExitStack

import concourse.bass as bass
import concourse.mybir as mybir
import concourse.tile as tile
from concourse import bacc
from concourse.bass_utils import run_bass_kernel_spmd

B, R, C, E, H, D, MS = 32, 128, 128, 256, 16, 16, 16
NCORES = 8
BL = B // NCORES  # batches per core: 4
TOK = BL * R      # 512
PTS = R * C       # 16384
KR = 18           # rhs rows: 16 dots + y-hi + y-res
BLOB = 2 * TOK + 2 * 512 + E  # xTr, xTc, WqP, WkP, Wv cols per e-half: 2304

FP32 = mybir.dt.float32
FP16 = mybir.dt.float16
FP8 = mybir.dt.float8e4
AF = mybir.ActivationFunctionType
ALU = mybir.AluOpType
DR = mybir.MatmulPerfMode.DoubleRow
E4NP = ml_dtypes.float8_e4m3


def build_kernel():
    nc = bacc.Bacc("TRN2", target_bir_lowering=False, debug=False,
                   num_devices=NCORES)

    blob_d = nc.dram_tensor("blob", [2, 128, BLOB], FP16,
                            kind="ExternalInput").ap()
    cost_d = nc.dram_tensor("cost8", [BL, 2, R, C], FP8,
                            kind="ExternalInput").ap()
    w1_d = nc.dram_tensor("W1DR", [KR, 512], FP8, kind="ExternalInput").ap()
    w2_d = nc.dram_tensor("W2L", [128, 16], FP16, kind="ExternalInput").ap()
    bc_d = nc.dram_tensor("bcol2", [128, 2], FP32, kind="ExternalInput").ap()
    x4d_d = nc.dram_tensor("x4d", [128, H, BL, C], FP8, kind="Internal").ap()
    out_d = nc.dram_tensor("out", [BL, R, H * D], FP32,
                           kind="ExternalOutput").ap()

    with tile.TileContext(nc) as tc, ExitStack() as ctx:
        const_p = ctx.enter_context(tc.tile_pool(name="const", bufs=1))
        big_p = ctx.enter_context(tc.tile_pool(name="big", bufs=1))
        qkv_p = ctx.enter_context(tc.tile_pool(name="qkv", bufs=1))
        x4_p = ctx.enter_context(tc.tile_pool(name="x4", bufs=1))
        rhs_p = ctx.enter_context(tc.tile_pool(name="rhs", bufs=2))
        rr_p = ctx.enter_context(tc.tile_pool(name="rr", bufs=4))
        wsb_p = ctx.enter_context(tc.tile_pool(name="wsb", bufs=2))
        fout_p = ctx.enter_context(tc.tile_pool(name="fout", bufs=1))
        small_p = ctx.enter_context(tc.tile_pool(name="small", bufs=4))
        # PSUM: psb 3x[128,1024] = 6 banks, psl2 1x[128,512] = 1, psa 1 = 8
        ps_big = ctx.enter_context(
            tc.tile_pool(name="psb", bufs=3, space="PSUM"))
        ps_l2 = ctx.enter_context(
            tc.tile_pool(name="psl2", bufs=1, space="PSUM"))
        ps_av = ctx.enter_context(
            tc.tile_pool(name="psa", bufs=1, space="PSUM"))

        # ---- packed fp16 input blob: 2 DMAs
        big = big_p.tile([128, 2 * BLOB], FP16)
        for eh in range(2):
            nc.sync.dma_start(big[:, eh * BLOB:(eh + 1) * BLOB], blob_d[eh])

        def blob_slice(eh, off, n):
            return big[:, eh * BLOB + off:eh * BLOB + off + n]

        xT = {"r": [blob_slice(eh, 0, TOK) for eh in range(2)],
              "c": [blob_slice(eh, TOK, TOK) for eh in range(2)]}
        wt16 = {"q": [blob_slice(eh, 2 * TOK, 512) for eh in range(2)],
                "k": [blob_slice(eh, 2 * TOK + 512, 512) for eh in range(2)],
                "v": [blob_slice(eh, 2 * TOK + 1024, E) for eh in range(2)]}

        # ---- small consts
        w1dr = const_p.tile([KR, 512], FP8)
        nc.sync.dma_start(w1dr[:], w1_d[:])
        w1v = w1dr[:].rearrange("p (h t m) -> p h t m", h=2, t=2)

        w2l = const_p.tile([128, 16], FP16)
        nc.sync.dma_start(w2l[:], w2_d[:])

        bcol2 = const_p.tile([128, 2], FP32)
        nc.sync.dma_start(bcol2[:], bc_d[:])

        # ---- projections: qT/kT per-head tiles [16, TOK] fp16
        qT, kT = [], []
        for proj, dst in (("q", qT), ("k", kT)):
            for mh in range(4):  # head-quad tiles (4 heads x 32 rows)
                ps = ps_big.tile([128, 1024], FP32, tag="psbig")
                for eh in range(2):
                    nc.tensor.matmul(
                        ps[:, 0:TOK],
                        wt16[proj][eh][:, mh * 128:(mh + 1) * 128],
                        xT["r" if proj == "q" else "c"][eh],
                        start=(eh == 0), stop=(eh == 1))
                quad = qkv_p.tile([96, TOK], FP16, tag=f"{proj}Q{mh}",
                                  name=f"{proj}Q{mh}")
                last = qkv_p.tile([16, TOK], FP16, tag=f"{proj}L{mh}",
                                  name=f"{proj}L{mh}")
                if mh % 2 == 0:
                    nc.scalar.copy(quad[:], ps[0:96, 0:TOK])
                    nc.vector.tensor_copy(last[:], ps[96:112, 0:TOK])
                else:
                    nc.vector.tensor_copy(quad[:], ps[0:96, 0:TOK])
                    nc.scalar.copy(last[:], ps[96:112, 0:TOK])
                for hh in range(4):
                    dst.append(quad[hh * 32:hh * 32 + 16, :] if hh < 3
                               else last[:])

        # ---- dots into one tile [r, (h, b, c)] fp8, then DRAM bounce
        x4big = x4_p.tile([128, H * BL * C], FP8, name="x4big")
        for h2 in range(H // 2):
            psd = ps_big.tile([128, 1024], FP32, tag="psbig")
            for hh in range(2):
                h = h2 * 2 + hh
                for b in range(BL):
                    nc.tensor.matmul(
                        psd[:, hh * 512 + b * 128:hh * 512 + (b + 1) * 128],
                        qT[h][:, b * 128:(b + 1) * 128],
                        kT[h][:, b * 128:(b + 1) * 128])
            dstv = x4big[:, h2 * 1024:(h2 + 1) * 1024]
            if h2 % 2 == 0:
                nc.scalar.copy(dstv, psd[:])
            else:
                nc.vector.tensor_copy(dstv, psd[:])
        nc.sync.dma_start(x4d_d[:], x4big[:])  # one dump DMA

        # ---- v natural [c, hd] fp16 interleaved with ones col -> vhat
        vhat = []
        for b in range(BL):
            vh = qkv_p.tile([128, 17 * H], FP16, tag=f"vhat{b}",
                            name=f"vhat{b}")
            vh3 = vh[:].rearrange("p (h x) -> p h x", h=H)
            nc.gpsimd.memset(vh3[:, :, 16:17], 1.0)
            ps = ps_big.tile([128, 1024], FP32, tag="psbig")
            for eh in range(2):
                nc.tensor.matmul(
                    ps[:, 0:E], xT["c"][eh][:, b * 128:(b + 1) * 128],
                    wt16["v"][eh], start=(eh == 0), stop=(eh == 1))
            nc.vector.tensor_copy(
                vh3[:, :, 0:16],
                ps[:, 0:E].rearrange("p (h x) -> p h x", h=H))
            vhat.append(vh)


        # ---- per (b): rhs gather (2 DMAs), layer1+relu, layer2, exp, AV
        fouts = fout_p.tile([128, BL * H * D], FP32, name="fouts")
        evac_ctr = [0]
        for b in range(BL):
            rhs = rhs_p.tile([KR, PTS], FP8, tag="rhs")
            nc.sync.dma_start(rhs[0:16, :], x4d_d[:, :, b, :].rearrange(
                "p h c -> h p c"))
            nc.sync.dma_start(rhs[16:18, :], cost_d[b])
            rhs_dup = rhs[:].rearrange("p (o n) -> p o n", o=1)

            for half in range(2):
                wsb = wsb_p.tile([128, 1024], FP16)
                for grp in range(2):
                    ps2 = ps_l2.tile([128, 512], FP32)
                    for ckq in range(8):  # 1024-pt chunk pairs
                        ckp = grp * 8 + ckq
                        rr = rr_p.tile([128, 1024], FP16, tag="rr")
                        ps1 = ps_big.tile([128, 1024], FP32, tag="psbig")
                        for u in range(2):
                            ck = ckp * 2 + u
                            nc.tensor.matmul(
                                ps1[:, u * 512:(u + 1) * 512], w1v[:, half],
                                rhs_dup[:, :, ck * 512:(ck + 1) * 512]
                                .broadcast_to((KR, 2, 512)),
                                perf_mode=DR)
                        i = evac_ctr[0]
                        evac_ctr[0] += 1
                        if (i * 33) % 64 < 33:  # interleaved ~66:62 ACT:DVE
                            nc.scalar.activation(
                                rr[:], ps1[:], AF.Relu,
                                bias=bcol2[:, half:half + 1])
                        else:
                            nc.vector.tensor_scalar(
                                rr[:], ps1[:], bcol2[:, half:half + 1],
                                0.0, ALU.add, ALU.max)
                        for s in range(8):  # layer2 per 128-pt subchunk
                            rloc = ckq * 8 + s
                            nc.tensor.matmul(
                                ps2[:, rloc * 8:rloc * 8 + 8],
                                rr[:, s * 128:(s + 1) * 128],
                                w2l[:, half * 8:(half + 1) * 8])
                    nc.scalar.activation(
                        wsb[:, grp * 512:(grp + 1) * 512], ps2[:], AF.Exp)

                # AV + normalize for the 8 heads of this half
                psa = ps_av.tile([128, 17 * 8], FP32)
                wsb4 = wsb[:].rearrange("p (g s h) -> p g s h", g=2, s=64)
                for hl in range(8):
                    h = half * 8 + hl
                    nc.tensor.matmul(
                        psa[:, hl * 17:(hl + 1) * 17],
                        wsb4[:, :, :, hl],
                        vhat[b][:, h * 17:(h + 1) * 17])
                rec = small_p.tile([128, 8], FP32, tag="rec")
                psa3 = psa[:].rearrange("p (x y) -> p x y", x=8)
                nc.vector.reciprocal(rec[:], psa3[:, :, 16])
                rec3 = rec[:].rearrange("p (h o) -> p h o", o=1)
                nc.vector.tensor_tensor(
                    fouts[:, b * 256 + half * 128:b * 256 + half * 128 + 128]
                    .rearrange("p (h x) -> p h x", h=8),
                    psa3[:, :, 0:16],
                    rec3.broadcast_to((128, 8, 16)),
                    ALU.mult)

        # one output DMA; DRAM side reordered to (r, b, e)
        nc.sync.dma_start(
            out_d[:].rearrange("b r e -> r b e"),
            fouts[:].rearrange("p (b e) -> p b e", b=BL))

    nc.compile()
    return nc


_cache = {}


def _prep_consts(Wq, Wk, Wv, m1w, m1b, m2w):
    a1 = m1w[:, 0, :]
    c1 = m1w[:, 1, :]
    w2 = m2w[:, :, 0]

    wq_s = (Wq * (1.0 / np.sqrt(D))).astype(np.float16)
    wk16 = Wk.astype(np.float16)
    wv16 = Wv.astype(np.float16)
    wq_pad = np.zeros((E, 2 * E), np.float16)
    wk_pad = np.zeros((E, 2 * E), np.float16)
    for h in range(H):
        wq_pad[:, h * 32:h * 32 + 16] = wq_s[:, h * 16:(h + 1) * 16]
        wk_pad[:, h * 32:h * 32 + 16] = wk16[:, h * 16:(h + 1) * 16]

    w1x = np.zeros((KR, 256), np.float32)
    w2l = np.zeros((128, 16), np.float16)
    bcol2 = np.zeros((128, 2), np.float32)
    for h in range(H):
        half, hl = h // 8, h % 8
        for m in range(MS):
            col = half * 128 + hl * 16 + m
            w1x[h, col] = a1[h, m]
            w1x[16, col] = c1[h, m]
            w1x[17, col] = c1[h, m]
            w2l[hl * 16 + m, half * 8 + hl] = w2[h, m]
            bcol2[hl * 16 + m, half] = m1b[h, m]

    w1hi = w1x.astype(E4NP)
    w1res = (w1x - w1hi.astype(np.float32)).astype(E4NP)
    w1dr = np.zeros((KR, 512), E4NP)
    for half in range(2):
        w1dr[:, half * 256:half * 256 + 128] = \
            w1hi[:, half * 128:(half + 1) * 128]
        w1dr[:, half * 256 + 128:half * 256 + 256] = \
            w1res[:, half * 128:(half + 1) * 128]
    return wq_pad, wk_pad, wv16, w1dr, w2l, bcol2


def kernel(**inputs):
    row_emb = np.asarray(inputs["row_emb"], dtype=np.float32)
    col_emb = np.asarray(inputs["col_emb"], dtype=np.float32)
    cost_mat = np.asarray(inputs["cost_mat"], dtype=np.float32)
    Wq = np.asarray(inputs["Wq"], dtype=np.float32)
    Wk = np.asarray(inputs["Wk"], dtype=np.float32)
    Wv = np.asarray(inputs["Wv"], dtype=np.float32)
    m1w = np.asarray(inputs["mix1_weight"], dtype=np.float32)
    m1b = np.asarray(inputs["mix1_bias"], dtype=np.float32)
    m2w = np.asarray(inputs["mix2_weight"], dtype=np.float32)

    if "nc" not in _cache:
        _cache["nc"] = build_kernel()
    nc = _cache["nc"]

    wq_pad, wk_pad, wv16, w1dr, w2l, bcol2 = _prep_consts(
        Wq, Wk, Wv, m1w, m1b, m2w)
    cost8 = cost_mat.astype(E4NP)
    cost8r = (cost_mat - cost8.astype(np.float32)).astype(E4NP)
    costc = np.stack([cost8, cost8r], axis=1)  # [B, 2, R, C]

    in_maps = []
    for i in range(NCORES):
        sl = slice(i * BL, (i + 1) * BL)
        xtr = np.ascontiguousarray(
            row_emb[sl].reshape(TOK, E).T).astype(np.float16)
        xtc = np.ascontiguousarray(
            col_emb[sl].reshape(TOK, E).T).astype(np.float16)
        blob = np.zeros((2, 128, BLOB), np.float16)
        for eh in range(2):
            rows = slice(eh * 128, (eh + 1) * 128)
            blob[eh, :, 0:TOK] = xtr[rows]
            blob[eh, :, TOK:2 * TOK] = xtc[rows]
            blob[eh, :, 2 * TOK:2 * TOK + 512] = wq_pad[rows]
            blob[eh, :, 2 * TOK + 512:2 * TOK + 1024] = wk_pad[rows]
            blob[eh, :, 2 * TOK + 1024:2 * TOK + 1280] = wv16[rows]
        in_maps.append({
            "blob": blob,
            "cost8": costc[sl],
            "W1DR": w1dr, "W2L": w2l, "bcol2": bcol2,
        })
    res = run_bass_kernel_spmd(nc, in_maps, list(range(NCORES)))
    out = np.concatenate([res.results[i]["out"] for i in range(NCORES)],
                         axis=0)
    return out.astype(np.float32)


# revision 3
# speedup vs baseline: 1.0347x; 1.0212x over previous
"""MixedScoreMultiHeadAttention Trainium2 kernel.

v3 + DMA-count reduction (the v3 profile showed 45+us of serialized HWDGE
issue overhead from 89 DMAs at ~625ns each):
  - all fp16 inputs packed host-side into one blob -> 2 input DMAs
  - dots staged through a DRAM bounce: one SBUF->DRAM dump, then ONE
    DRAM->SBUF gather per batch (h-major on both sides, so ordering works;
    a direct SBUF->SBUF multi-row gather iterates the source r-major and
    scrambles)
  - cost hi+res combined into one [BL, 2, R, C] tensor -> 1 DMA per batch
  - single fouts tile + one output DMA (DRAM side reordered r-major)
Also: projection/dot evacuations split across ACT/DVE so ACT is not idle
during warmup.
"""
import sys

sys.path.insert(0, "/opt/trn_rl_repo")

import numpy as np
import ml_dtypes
from contextlib import ExitStack

import concourse.bass as bass
import concourse.mybir as mybir
import concourse.tile as tile
from concourse import bacc
from concourse.bass_utils import run_bass_kernel_spmd

B, R, C, E, H, D, MS = 32, 128, 128, 256, 16, 16, 16
NCORES = 8
BL = B // NCORES  # batches per core: 4
TOK = BL * R      # 512
PTS = R * C       # 16384
KR = 18           # rhs rows: 16 dots + y-hi + y-res
BLOB = 2 * TOK + 2 * 512 + E  # xTr, xTc, WqP, WkP, Wv cols per e-half: 2304

FP32 = mybir.dt.float32
FP16 = mybir.dt.float16
FP8 = mybir.dt.float8e4
AF = mybir.ActivationFunctionType
ALU = mybir.AluOpType
DR = mybir.MatmulPerfMode.DoubleRow
E4NP = ml_dtypes.float8_e4m3


def build_kernel():
    nc = bacc.Bacc("TRN2", target_bir_lowering=False, debug=False,
                   num_devices=NCORES)

    blob_d = nc.dram_tensor("blob", [2, 128, BLOB], FP16,
                            kind="ExternalInput").ap()
    cost_d = nc.dram_tensor("cost8", [BL, 2, R, C], FP8,
                            kind="ExternalInput").ap()
    w1_d = nc.dram_tensor("W1DR", [KR, 512], FP8, kind="ExternalInput").ap()
    w2_d = nc.dram_tensor("W2L", [128, 16], FP16, kind="ExternalInput").ap()
    bc_d = nc.dram_tensor("bcol2", [128, 2], FP32, kind="ExternalInput").ap()
    x4d_d = nc.dram_tensor("x4d", [128, H, BL, C], FP8, kind="Internal").ap()
    out_d = nc.dram_tensor("out", [BL, R, H * D], FP32,
                           kind="ExternalOutput").ap()

    with tile.TileContext(nc) as tc, ExitStack() as ctx:
        const_p = ctx.enter_context(tc.tile_pool(name="const", bufs=1))
        big_p = ctx.enter_context(tc.tile_pool(name="big", bufs=1))
        qkv_p = ctx.enter_context(tc.tile_pool(name="qkv", bufs=1))
        x4_p = ctx.enter_context(tc.tile_pool(name="x4", bufs=1))
        rhs_p = ctx.enter_context(tc.tile_pool(name="rhs", bufs=2))
        rr_p = ctx.enter_context(tc.tile_pool(name="rr", bufs=4))
        wsb_p = ctx.enter_context(tc.tile_pool(name="wsb", bufs=2))
        fout_p = ctx.enter_context(tc.tile_pool(name="fout", bufs=1))
        small_p = ctx.enter_context(tc.tile_pool(name="small", bufs=4))
        # PSUM: psb 3x[128,1024] = 6 banks, psl2 1x[128,512] = 1, psa 1 = 8
        ps_big = ctx.enter_context(
            tc.tile_pool(name="psb", bufs=3, space="PSUM"))
        ps_l2 = ctx.enter_context(
            tc.tile_pool(name="psl2", bufs=1, space="PSUM"))
        ps_av = ctx.enter_context(
            tc.tile_pool(name="psa", bufs=1, space="PSUM"))

        # ---- packed fp16 input blob: 2 DMAs
        big = big_p.tile([128, 2 * BLOB], FP16)
        for eh in range(2):
            nc.sync.dma_start(big[:, eh * BLOB:(eh + 1) * BLOB], blob_d[eh])

        def blob_slice(eh, off, n):
            return big[:, eh * BLOB + off:eh * BLOB + off + n]

        xT = {"r": [blob_slice(eh, 0, TOK) for eh in range(2)],
              "c": [blob_slice(eh, TOK, TOK) for eh in range(2)]}
        wt16 = {"q": [blob_slice(eh, 2 * TOK, 512) for eh in range(2)],
                "k": [blob_slice(eh, 2 * TOK + 512, 512) for eh in range(2)],
                "v": [blob_slice(eh, 2 * TOK + 1024, E) for eh in range(2)]}

        # ---- small consts
        w1dr = const_p.tile([KR, 512], FP8)
        nc.sync.dma_start(w1dr[:], w1_d[:])
        w1v = w1dr[:].rearrange("p (h t m) -> p h t m", h=2, t=2)

        w2l = const_p.tile([128, 16], FP16)
        nc.sync.dma_start(w2l[:], w2_d[:])

        bcol2 = const_p.tile([128, 2], FP32)
        nc.sync.dma_start(bcol2[:], bc_d[:])

        # ---- projections interleaved with dots: each head-quad's dot
        # products start as soon as its q/k land; b0's rhs rows stream out
        # via direct DMAs while later quads still compute
        qT, kT = [], []
        x4big = x4_p.tile([128, H * BL * C], FP8, name="x4big")
        rhs0 = rhs_p.tile([KR, PTS], FP8, tag="rhs")
        nc.sync.dma_start(rhs0[16:18, :], cost_d[0])
        ectr = 0
        for mh in range(4):
            for proj, dst in (("q", qT), ("k", kT)):
                ps = ps_big.tile([128, 1024], FP32, tag="psbig")
                for eh in range(2):
                    nc.tensor.matmul(
                        ps[:, 0:TOK],
                        wt16[proj][eh][:, mh * 128:(mh + 1) * 128],
                        xT["r" if proj == "q" else "c"][eh],
                        start=(eh == 0), stop=(eh == 1))
                quad = qkv_p.tile([96, TOK], FP16, tag=f"{proj}Q{mh}",
                                  name=f"{proj}Q{mh}")
                last = qkv_p.tile([16, TOK], FP16, tag=f"{proj}L{mh}",
                                  name=f"{proj}L{mh}")
                if ectr % 2 == 0:
                    nc.scalar.copy(quad[:], ps[0:96, 0:TOK])
                    nc.vector.tensor_copy(last[:], ps[96:112, 0:TOK])
                else:
                    nc.vector.tensor_copy(quad[:], ps[0:96, 0:TOK])
                    nc.scalar.copy(last[:], ps[96:112, 0:TOK])
                ectr += 1
                for hh in range(4):
                    dst.append(quad[hh * 32:hh * 32 + 16, :] if hh < 3
                               else last[:])
            for h2p in range(2):  # dots pairs for this quad
                h2 = 2 * mh + h2p
                psd = ps_big.tile([128, 1024], FP32, tag="psbig")
                for hh in range(2):
                    h = h2 * 2 + hh
                    for b in range(BL):
                        nc.tensor.matmul(
                            psd[:, hh * 512 + b * 128:
                                hh * 512 + (b + 1) * 128],
                            qT[h][:, b * 128:(b + 1) * 128],
                            kT[h][:, b * 128:(b + 1) * 128])
                dstv = x4big[:, h2 * 1024:(h2 + 1) * 1024]
                if ectr % 2 == 0:
                    nc.scalar.copy(dstv, psd[:])
                else:
                    nc.vector.tensor_copy(dstv, psd[:])
                ectr += 1
                for hh in range(2):  # b0 direct row gathers
                    h = h2 * 2 + hh
                    off = h2 * 1024 + hh * 512
                    nc.sync.dma_start(rhs0[h:h + 1, :],
                                      x4big[:, off:off + 128])
        nc.sync.dma_start(x4d_d[:], x4big[:])  # dump DMA for b1..b3

        # ---- v natural [c, hd] fp16 interleaved with ones col -> vhat
        vhat = []
        for b in range(BL):
            vh = qkv_p.tile([128, 17 * H], FP16, tag=f"vhat{b}",
                            name=f"vhat{b}")
            vh3 = vh[:].rearrange("p (h x) -> p h x", h=H)
            nc.gpsimd.memset(vh3[:, :, 16:17], 1.0)
            ps = ps_av.tile([128, E], FP32, tag="psv")
            for eh in range(2):
                nc.tensor.matmul(
                    ps[:], xT["c"][eh][:, b * 128:(b + 1) * 128],
                    wt16["v"][eh], start=(eh == 0), stop=(eh == 1))
            if b % 2 == 0:
                nc.scalar.copy(
                    vh3[:, :, 0:16],
                    ps[:].rearrange("p (h x) -> p h x", h=H))
            else:
                nc.vector.tensor_copy(
                    vh3[:, :, 0:16],
                    ps[:].rearrange("p (h x) -> p h x", h=H))
            vhat.append(vh)


        # ---- per (b): rhs gather (2 DMAs), layer1+relu, layer2, exp, AV
        fouts = fout_p.tile([128, BL * H * D], FP32, name="fouts")
        evac_ctr = [0]
        for b in range(BL):
            if b == 0:
                rhs = rhs0
            else:
                rhs = rhs_p.tile([KR, PTS], FP8, tag="rhs")
                nc.sync.dma_start(rhs[0:16, :], x4d_d[:, :, b, :].rearrange(
                    "p h c -> h p c"))
                nc.sync.dma_start(rhs[16:18, :], cost_d[b])
            rhs_dup = rhs[:].rearrange("p (o n) -> p o n", o=1)

            for half in range(2):
                wsb = wsb_p.tile([128, 1024], FP16)
                for grp in range(2):
                    ps2 = ps_l2.tile([128, 512], FP32)
                    for ckq in range(8):  # 1024-pt chunk pairs
                        ckp = grp * 8 + ckq
                        rr = rr_p.tile([128, 1024], FP16, tag="rr")
                        ps1 = ps_big.tile([128, 1024], FP32, tag="psbig")
                        for u in range(2):
                            ck = ckp * 2 + u
                            nc.tensor.matmul(
                                ps1[:, u * 512:(u + 1) * 512], w1v[:, half],
                                rhs_dup[:, :, ck * 512:(ck + 1) * 512]
                                .broadcast_to((KR, 2, 512)),
                                perf_mode=DR)
                        i = evac_ctr[0]
                        evac_ctr[0] += 1
                        if (i * 33) % 64 < 33:  # interleaved ~66:62 ACT:DVE
                            nc.scalar.activation(
                                rr[:], ps1[:], AF.Relu,
                                bias=bcol2[:, half:half + 1])
                        else:
                            nc.vector.tensor_scalar(
                                rr[:], ps1[:], bcol2[:, half:half + 1],
                                0.0, ALU.add, ALU.max)
                        for s in range(8):  # layer2 per 128-pt subchunk
                            rloc = ckq * 8 + s
                            nc.tensor.matmul(
                                ps2[:, rloc * 8:rloc * 8 + 8],
                                rr[:, s * 128:(s + 1) * 128],
                                w2l[:, half * 8:(half + 1) * 8])
                    nc.scalar.activation(
                        wsb[:, grp * 512:(grp + 1) * 512], ps2[:], AF.Exp)

                # AV + normalize for the 8 heads of this half
                psa = ps_av.tile([128, E], FP32, tag="psv")
                wsb4 = wsb[:].rearrange("p (g s h) -> p g s h", g=2, s=64)
                for hl in range(8):
                    h = half * 8 + hl
                    nc.tensor.matmul(
                        psa[:, hl * 17:(hl + 1) * 17],
                        wsb4[:, :, :, hl],
                        vhat[b][:, h * 17:(h + 1) * 17])
                rec = small_p.tile([128, 8], FP32, tag="rec")
                psa3 = psa[:, 0:136].rearrange("p (x y) -> p x y", x=8)
                nc.vector.reciprocal(rec[:], psa3[:, :, 16])
                rec3 = rec[:].rearrange("p (h o) -> p h o", o=1)
                nc.vector.tensor_tensor(
                    fouts[:, b * 256 + half * 128:b * 256 + half * 128 + 128]
                    .rearrange("p (h x) -> p h x", h=8),
                    psa3[:, :, 0:16],
                    rec3.broadcast_to((128, 8, 16)),
                    ALU.mult)
            nc.sync.dma_start(out_d[b],
                              fouts[:, b * 256:(b + 1) * 256])


    nc.compile()
    return nc


_cache = {}


def _prep_consts(Wq, Wk, Wv, m1w, m1b, m2w):
    a1 = m1w[:, 0, :]
    c1 = m1w[:, 1, :]
    w2 = m2w[:, :, 0]

    wq_s = (Wq * (1.0 / np.sqrt(D))).astype(np.float16)
    wk16 = Wk.astype(np.float16)
    wv16 = Wv.astype(np.float16)
    wq_pad = np.zeros((E, 2 * E), np.float16)
    wk_pad = np.zeros((E, 2 * E), np.float16)
    for h in range(H):
        wq_pad[:, h * 32:h * 32 + 16] = wq_s[:, h * 16:(h + 1) * 16]
        wk_pad[:, h * 32:h * 32 + 16] = wk16[:, h * 16:(h + 1) * 16]

    w1x = np.zeros((KR, 256), np.float32)
    w2l = np.zeros((128, 16), np.float16)
    bcol2 = np.zeros((128, 2), np.float32)
    for h in range(H):
        half, hl = h // 8, h % 8
        for m in range(MS):
            col = half * 128 + hl * 16 + m
            w1x[h, col] = a1[h, m]
            w1x[16, col] = c1[h, m]
            w1x[17, col] = c1[h, m]
            w2l[hl * 16 + m, half * 8 + hl] = w2[h, m]
            bcol2[hl * 16 + m, half] = m1b[h, m]

    w1hi = w1x.astype(E4NP)
    w1res = (w1x - w1hi.astype(np.float32)).astype(E4NP)
    w1dr = np.zeros((KR, 512), E4NP)
    for half in range(2):
        w1dr[:, half * 256:half * 256 + 128] = \
            w1hi[:, half * 128:(half + 1) * 128]
        w1dr[:, half * 256 + 128:half * 256 + 256] = \
            w1res[:, half * 128:(half + 1) * 128]
    return wq_pad, wk_pad, wv16, w1dr, w2l, bcol2


def kernel(**inputs):
    row_emb = np.asarray(inputs["row_emb"], dtype=np.float32)
    col_emb = np.asarray(inputs["col_emb"], dtype=np.float32)
    cost_mat = np.asarray(inputs["cost_mat"], dtype=np.float32)
    Wq = np.asarray(inputs["Wq"], dtype=np.float32)
    Wk = np.asarray(inputs["Wk"], dtype=np.float32)
    Wv = np.asarray(inputs["Wv"], dtype=np.float32)
    m1w = np.asarray(inputs["mix1_weight"], dtype=np.float32)
    m1b = np.asarray(inputs["mix1_bias"], dtype=np.float32)
    m2w = np.asarray(inputs["mix2_weight"], dtype=np.float32)

    if "nc" not in _cache:
        _cache["nc"] = build_kernel()
    nc = _cache["nc"]

    wq_pad, wk_pad, wv16, w1dr, w2l, bcol2 = _prep_consts(
        Wq, Wk, Wv, m1w, m1b, m2w)
    cost8 = cost_mat.astype(E4NP)
    cost8r = (cost_mat - cost8.astype(np.float32)).astype(E4NP)
    costc = np.stack([cost8, cost8r], axis=1)  # [B, 2, R, C]

    in_maps = []
    for i in range(NCORES):
        sl = slice(i * BL, (i + 1) * BL)
        xtr = np.ascontiguousarray(
            row_emb[sl].reshape(TOK, E).T).astype(np.float16)
        xtc = np.ascontiguousarray(
            col_emb[sl].reshape(TOK, E).T).astype(np.float16)
        blob = np.zeros((2, 128, BLOB), np.float16)
        for eh in range(2):
            rows = slice(eh * 128, (eh + 1) * 128)
            blob[eh, :, 0:TOK] = xtr[rows]
            blob[eh, :, TOK:2 * TOK] = xtc[rows]
            blob[eh, :, 2 * TOK:2 * TOK + 512] = wq_pad[rows]
            blob[eh, :, 2 * TOK + 512:2 * TOK + 1024] = wk_pad[rows]
            blob[eh, :, 2 * TOK + 1024:2 * TOK + 1280] = wv16[rows]
        in_maps.append({
            "blob": blob,
            "cost8": costc[sl],
            "W1DR": w1dr, "W2L": w2l, "bcol2": bcol2,
        })
    res = run_bass_kernel_spmd(nc, in_maps, list(range(NCORES)))
    out = np.concatenate([res.results[i]["out"] for i in range(NCORES)],
                         axis=0)
    return out.astype(np.float32)


# revision 4
# speedup vs baseline: 1.0404x; 1.0056x over previous
"""MixedScoreMultiHeadAttention Trainium2 kernel.

v3 + DMA-count reduction (the v3 profile showed 45+us of serialized HWDGE
issue overhead from 89 DMAs at ~625ns each):
  - all fp16 inputs packed host-side into one blob -> 2 input DMAs
  - dots staged through a DRAM bounce: one SBUF->DRAM dump, then ONE
    DRAM->SBUF gather per batch (h-major on both sides, so ordering works;
    a direct SBUF->SBUF multi-row gather iterates the source r-major and
    scrambles)
  - cost hi+res combined into one [BL, 2, R, C] tensor -> 1 DMA per batch
  - single fouts tile + one output DMA (DRAM side reordered r-major)
Also: projection/dot evacuations split across ACT/DVE so ACT is not idle
during warmup.
"""
import sys

sys.path.insert(0, "/opt/trn_rl_repo")

import numpy as np
import ml_dtypes
from contextlib import ExitStack

import concourse.bass as bass
import concourse.mybir as mybir
import concourse.tile as tile
from concourse import bacc
from concourse.bass_utils import run_bass_kernel_spmd

B, R, C, E, H, D, MS = 32, 128, 128, 256, 16, 16, 16
NCORES = 8
BL = B // NCORES  # batches per core: 4
TOK = BL * R      # 512
PTS = R * C       # 16384
KR = 18           # rhs rows: 16 dots + y-hi + y-res
BLOB = 2 * TOK + 2 * 512 + E  # xTr, xTc, WqP, WkP, Wv cols per e-half: 2304

FP32 = mybir.dt.float32
FP16 = mybir.dt.float16
FP8 = mybir.dt.float8e4
AF = mybir.ActivationFunctionType
ALU = mybir.AluOpType
DR = mybir.MatmulPerfMode.DoubleRow
E4NP = ml_dtypes.float8_e4m3


def build_kernel():
    nc = bacc.Bacc("TRN2", target_bir_lowering=False, debug=False,
                   num_devices=NCORES)

    blob_d = nc.dram_tensor("blob", [2, 128, BLOB], FP16,
                            kind="ExternalInput").ap()
    cost_d = nc.dram_tensor("cost8", [BL, 2, R, C], FP8,
                            kind="ExternalInput").ap()
    w1_d = nc.dram_tensor("W1DR", [KR, 512], FP8, kind="ExternalInput").ap()
    w2_d = nc.dram_tensor("W2L", [128, 16], FP16, kind="ExternalInput").ap()
    bc_d = nc.dram_tensor("bcol2", [128, 2], FP32, kind="ExternalInput").ap()
    x4d_d = nc.dram_tensor("x4d", [128, H, BL, C], FP8, kind="Internal").ap()
    out_d = nc.dram_tensor("out", [BL, R, H * D], FP32,
                           kind="ExternalOutput").ap()

    with tile.TileContext(nc) as tc, ExitStack() as ctx:
        const_p = ctx.enter_context(tc.tile_pool(name="const", bufs=1))
        big_p = ctx.enter_context(tc.tile_pool(name="big", bufs=1))
        qkv_p = ctx.enter_context(tc.tile_pool(name="qkv", bufs=1))
        x4_p = ctx.enter_context(tc.tile_pool(name="x4", bufs=1))
        rhs_p = ctx.enter_context(tc.tile_pool(name="rhs", bufs=2))
        rr_p = ctx.enter_context(tc.tile_pool(name="rr", bufs=4))
        wsb_p = ctx.enter_context(tc.tile_pool(name="wsb", bufs=2))
        fout_p = ctx.enter_context(tc.tile_pool(name="fout", bufs=1))
        small_p = ctx.enter_context(tc.tile_pool(name="small", bufs=4))
        # PSUM: psb 3x[128,1024] = 6 banks, psl2 1x[128,512] = 1, psa 1 = 8
        ps_big = ctx.enter_context(
            tc.tile_pool(name="psb", bufs=3, space="PSUM"))
        ps_l2 = ctx.enter_context(
            tc.tile_pool(name="psl2", bufs=1, space="PSUM"))
        ps_av = ctx.enter_context(
            tc.tile_pool(name="psa", bufs=1, space="PSUM"))

        # ---- packed fp16 input blob: 2 DMAs
        big = big_p.tile([128, 2 * BLOB], FP16)
        for eh in range(2):
            nc.sync.dma_start(big[:, eh * BLOB:(eh + 1) * BLOB], blob_d[eh])

        def blob_slice(eh, off, n):
            return big[:, eh * BLOB + off:eh * BLOB + off + n]

        xT = {"r": [blob_slice(eh, 0, TOK) for eh in range(2)],
              "c": [blob_slice(eh, TOK, TOK) for eh in range(2)]}
        wt16 = {"q": [blob_slice(eh, 2 * TOK, 512) for eh in range(2)],
                "k": [blob_slice(eh, 2 * TOK + 512, 512) for eh in range(2)],
                "v": [blob_slice(eh, 2 * TOK + 1024, E) for eh in range(2)]}

        # ---- small consts
        w1dr = const_p.tile([KR, 512], FP8)
        nc.sync.dma_start(w1dr[:], w1_d[:])
        w1v = w1dr[:].rearrange("p (h t m) -> p h t m", h=2, t=2)

        w2l = const_p.tile([128, 16], FP16)
        nc.sync.dma_start(w2l[:], w2_d[:])

        bcol2 = const_p.tile([128, 2], FP32)
        nc.sync.dma_start(bcol2[:], bc_d[:])

        # ---- projections interleaved with dots: each head-quad's dot
        # products start as soon as its q/k land; b0's rhs rows stream out
        # via direct DMAs while later quads still compute
        qT, kT = [], []
        x4big = x4_p.tile([128, H * BL * C], FP8, name="x4big")
        rhs0 = rhs_p.tile([KR, PTS], FP8, tag="rhs")
        nc.sync.dma_start(rhs0[16:18, :], cost_d[0])
        ectr = 0
        for mh in range(4):
            for proj, dst in (("q", qT), ("k", kT)):
                ps = ps_big.tile([128, 1024], FP32, tag="psbig")
                for eh in range(2):
                    nc.tensor.matmul(
                        ps[:, 0:TOK],
                        wt16[proj][eh][:, mh * 128:(mh + 1) * 128],
                        xT["r" if proj == "q" else "c"][eh],
                        start=(eh == 0), stop=(eh == 1))
                quad = qkv_p.tile([96, TOK], FP16, tag=f"{proj}Q{mh}",
                                  name=f"{proj}Q{mh}")
                last = qkv_p.tile([16, TOK], FP16, tag=f"{proj}L{mh}",
                                  name=f"{proj}L{mh}")
                if ectr % 2 == 0:
                    nc.scalar.copy(quad[:], ps[0:96, 0:TOK])
                    nc.vector.tensor_copy(last[:], ps[96:112, 0:TOK])
                else:
                    nc.vector.tensor_copy(quad[:], ps[0:96, 0:TOK])
                    nc.scalar.copy(last[:], ps[96:112, 0:TOK])
                ectr += 1
                for hh in range(4):
                    dst.append(quad[hh * 32:hh * 32 + 16, :] if hh < 3
                               else last[:])
            for h2p in range(2):  # dots pairs for this quad
                h2 = 2 * mh + h2p
                psd = ps_big.tile([128, 1024], FP32, tag="psbig")
                for hh in range(2):
                    h = h2 * 2 + hh
                    for b in range(BL):
                        nc.tensor.matmul(
                            psd[:, hh * 512 + b * 128:
                                hh * 512 + (b + 1) * 128],
                            qT[h][:, b * 128:(b + 1) * 128],
                            kT[h][:, b * 128:(b + 1) * 128])
                dstv = x4big[:, h2 * 1024:(h2 + 1) * 1024]
                if ectr % 2 == 0:
                    nc.scalar.copy(dstv, psd[:])
                else:
                    nc.vector.tensor_copy(dstv, psd[:])
                ectr += 1
                for hh in range(2):  # b0 direct row gathers
                    h = h2 * 2 + hh
                    off = h2 * 1024 + hh * 512
                    nc.sync.dma_start(rhs0[h:h + 1, :],
                                      x4big[:, off:off + 128])
        nc.sync.dma_start(x4d_d[:], x4big[:])  # dump DMA for b1..b3

        # ---- v natural [c, hd] fp16 interleaved with ones col -> vhat
        vhat = []
        for b in range(BL):
            vh = qkv_p.tile([128, 17 * H], FP16, tag=f"vhat{b}",
                            name=f"vhat{b}")
            vh3 = vh[:].rearrange("p (h x) -> p h x", h=H)
            nc.gpsimd.memset(vh3[:, :, 16:17], 1.0)
            ps = ps_av.tile([128, 2 * 136], FP32, tag="psv")
            for eh in range(2):
                nc.tensor.matmul(
                    ps[:, 0:E], xT["c"][eh][:, b * 128:(b + 1) * 128],
                    wt16["v"][eh], start=(eh == 0), stop=(eh == 1))
            if b % 2 == 0:
                nc.scalar.copy(
                    vh3[:, :, 0:16],
                    ps[:, 0:E].rearrange("p (h x) -> p h x", h=H))
            else:
                nc.vector.tensor_copy(
                    vh3[:, :, 0:16],
                    ps[:, 0:E].rearrange("p (h x) -> p h x", h=H))
            vhat.append(vh)


        # ---- per (b): rhs gather (2 DMAs), layer1+relu, layer2, exp, AV
        fouts = fout_p.tile([128, BL * H * D], FP32, name="fouts")
        evac_ctr = [0]
        for b in range(BL):
            if b == 0:
                rhs = rhs0
            else:
                rhs = rhs_p.tile([KR, PTS], FP8, tag="rhs")
                nc.sync.dma_start(rhs[0:16, :], x4d_d[:, :, b, :].rearrange(
                    "p h c -> h p c"))
                nc.sync.dma_start(rhs[16:18, :], cost_d[b])
            rhs_dup = rhs[:].rearrange("p (o n) -> p o n", o=1)

            psa = ps_av.tile([128, 2 * 136], FP32, tag="psv")
            for half in range(2):
                wsb = wsb_p.tile([128, 1024], FP16)
                for grp in range(2):
                    ps2 = ps_l2.tile([128, 512], FP32)
                    for ckq in range(8):  # 1024-pt chunk pairs
                        ckp = grp * 8 + ckq
                        rr = rr_p.tile([128, 1024], FP16, tag="rr")
                        ps1 = ps_big.tile([128, 1024], FP32, tag="psbig")
                        for u in range(2):
                            ck = ckp * 2 + u
                            nc.tensor.matmul(
                                ps1[:, u * 512:(u + 1) * 512], w1v[:, half],
                                rhs_dup[:, :, ck * 512:(ck + 1) * 512]
                                .broadcast_to((KR, 2, 512)),
                                perf_mode=DR)
                        i = evac_ctr[0]
                        evac_ctr[0] += 1
                        if (i * 65) % 128 < 65:  # interleaved 65:63 ACT:DVE
                            nc.scalar.activation(
                                rr[:], ps1[:], AF.Relu,
                                bias=bcol2[:, half:half + 1])
                        else:
                            nc.vector.tensor_scalar(
                                rr[:], ps1[:], bcol2[:, half:half + 1],
                                0.0, ALU.add, ALU.max)
                        for s in range(8):  # layer2 per 128-pt subchunk
                            rloc = ckq * 8 + s
                            nc.tensor.matmul(
                                ps2[:, rloc * 8:rloc * 8 + 8],
                                rr[:, s * 128:(s + 1) * 128],
                                w2l[:, half * 8:(half + 1) * 8])
                    nc.scalar.activation(
                        wsb[:, grp * 512:(grp + 1) * 512], ps2[:], AF.Exp)

                # AV for the 8 heads of this half
                wsb4 = wsb[:].rearrange("p (g s h) -> p g s h", g=2, s=64)
                for hl in range(8):
                    h = half * 8 + hl
                    nc.tensor.matmul(
                        psa[:, half * 136 + hl * 17:
                            half * 136 + (hl + 1) * 17],
                        wsb4[:, :, :, hl],
                        vhat[b][:, h * 17:(h + 1) * 17])
            rec = small_p.tile([128, 16], FP32, tag="rec")
            psa4 = psa[:].rearrange("p (a x y) -> p a x y", a=2, x=8)
            nc.vector.reciprocal(rec[:], psa4[:, :, :, 16])
            nc.vector.tensor_tensor(
                fouts[:, b * 256:(b + 1) * 256]
                .rearrange("p (a x e) -> p a x e", a=2, x=8),
                psa4[:, :, :, 0:16],
                rec[:].rearrange("p (a x o) -> p a x o", a=2, o=1)
                .broadcast_to((128, 2, 8, 16)),
                ALU.mult)
            nc.sync.dma_start(out_d[b],
                              fouts[:, b * 256:(b + 1) * 256])


    nc.compile()
    return nc


_cache = {}


def _prep_consts(Wq, Wk, Wv, m1w, m1b, m2w):
    a1 = m1w[:, 0, :]
    c1 = m1w[:, 1, :]
    w2 = m2w[:, :, 0]

    wq_s = (Wq * (1.0 / np.sqrt(D))).astype(np.float16)
    wk16 = Wk.astype(np.float16)
    wv16 = Wv.astype(np.float16)
    wq_pad = np.zeros((E, 2 * E), np.float16)
    wk_pad = np.zeros((E, 2 * E), np.float16)
    for h in range(H):
        wq_pad[:, h * 32:h * 32 + 16] = wq_s[:, h * 16:(h + 1) * 16]
        wk_pad[:, h * 32:h * 32 + 16] = wk16[:, h * 16:(h + 1) * 16]

    w1x = np.zeros((KR, 256), np.float32)
    w2l = np.zeros((128, 16), np.float16)
    bcol2 = np.zeros((128, 2), np.float32)
    for h in range(H):
        half, hl = h // 8, h % 8
        for m in range(MS):
            col = half * 128 + hl * 16 + m
            w1x[h, col] = a1[h, m]
            w1x[16, col] = c1[h, m]
            w1x[17, col] = c1[h, m]
            w2l[hl * 16 + m, half * 8 + hl] = w2[h, m]
            bcol2[hl * 16 + m, half] = m1b[h, m]

    w1hi = w1x.astype(E4NP)
    w1res = (w1x - w1hi.astype(np.float32)).astype(E4NP)
    w1dr = np.zeros((KR, 512), E4NP)
    for half in range(2):
        w1dr[:, half * 256:half * 256 + 128] = \
            w1hi[:, half * 128:(half + 1) * 128]
        w1dr[:, half * 256 + 128:half * 256 + 256] = \
            w1res[:, half * 128:(half + 1) * 128]
    return wq_pad, wk_pad, wv16, w1dr, w2l, bcol2


def kernel(**inputs):
    row_emb = np.asarray(inputs["row_emb"], dtype=np.float32)
    col_emb = np.asarray(inputs["col_emb"], dtype=np.float32)
    cost_mat = np.asarray(inputs["cost_mat"], dtype=np.float32)
    Wq = np.asarray(inputs["Wq"], dtype=np.float32)
    Wk = np.asarray(inputs["Wk"], dtype=np.float32)
    Wv = np.asarray(inputs["Wv"], dtype=np.float32)
    m1w = np.asarray(inputs["mix1_weight"], dtype=np.float32)
    m1b = np.asarray(inputs["mix1_bias"], dtype=np.float32)
    m2w = np.asarray(inputs["mix2_weight"], dtype=np.float32)

    if "nc" not in _cache:
        _cache["nc"] = build_kernel()
    nc = _cache["nc"]

    wq_pad, wk_pad, wv16, w1dr, w2l, bcol2 = _prep_consts(
        Wq, Wk, Wv, m1w, m1b, m2w)
    cost8 = cost_mat.astype(E4NP)
    cost8r = (cost_mat - cost8.astype(np.float32)).astype(E4NP)
    costc = np.stack([cost8, cost8r], axis=1)  # [B, 2, R, C]

    in_maps = []
    for i in range(NCORES):
        sl = slice(i * BL, (i + 1) * BL)
        xtr = np.ascontiguousarray(
            row_emb[sl].reshape(TOK, E).T).astype(np.float16)
        xtc = np.ascontiguousarray(
            col_emb[sl].reshape(TOK, E).T).astype(np.float16)
        blob = np.zeros((2, 128, BLOB), np.float16)
        for eh in range(2):
            rows = slice(eh * 128, (eh + 1) * 128)
            blob[eh, :, 0:TOK] = xtr[rows]
            blob[eh, :, TOK:2 * TOK] = xtc[rows]
            blob[eh, :, 2 * TOK:2 * TOK + 512] = wq_pad[rows]
            blob[eh, :, 2 * TOK + 512:2 * TOK + 1024] = wk_pad[rows]
            blob[eh, :, 2 * TOK + 1024:2 * TOK + 1280] = wv16[rows]
        in_maps.append({
            "blob": blob,
            "cost8": costc[sl],
            "W1DR": w1dr, "W2L": w2l, "bcol2": bcol2,
        })
    res = run_bass_kernel_spmd(nc, in_maps, list(range(NCORES)))
    out = np.concatenate([res.results[i]["out"] for i in range(NCORES)],
                         axis=0)
    return out.astype(np.float32)
